# revision 44
# baseline (speedup 1.0000x reference)
"""GNN message-passing (nn_Net_4612794876089) Trainium2 kernel, v2.

Math (per batch b):
  y = sum_k neighbor[b,k,0,:]                     (F,)
  a[g,f] = x_g*y_f + y_g*x_f                      rank-2, symmetric
  S = sign(a)*sqrt(|a|)
  denom[g] = sum_f |S[g,f]| + 1e-7                (S symmetric)
  layer(h): out[c,f] = sum_g S[g,f] * (hraw[g,c]/denom[g])
  BN (global stats over (B,F) per channel) + softsign, twice; classifier.

v2 design (vs stash-reload baseline):
  * a is generated on DVE in bf16 via a = y_g * (x_f + (x_g/y_g)*y_f):
    one scalar_tensor_tensor for inner = x_f + u_g*y_f, then the y_g
    factor rides the ARS activation's per-partition `scale`, so the whole
    sgnroot pipeline is 3 wide DVE ops + 1 scalar op per [128,F] tile:
      inner (stt 4x), r = ARS(inner*y_g), S = inner*y_g*r (stt 4x),
      |S| + rowsum via stt(mult,max,accum_out) (4x).
  * layer-1 contraction streams bf16 S through the PE; S is also cast to
    fp8e4m3 via gpsimd cast-DMAs and kept fully SBUF-resident (16 MB),
    so layer 2 runs DoubleRow fp8 matmuls (2 k-tiles/partition, 0.5
    cyc/col) with no HBM stash at all.
  * BN transposes are batched [8,128]->[128,8] (all 4 batches at once),
    classifier is restructured as [128,4bat] x [128,47] accumulation.
  * hd vectors are scaled by 4096 (folded into 1/denom) so fp8 weights
    stay in e4m3's normal range; the evacuation rescales by 1/4096.
"""
import os
import sys
import numpy as np

sys.path.insert(0, "/opt/trn_rl_repo")

B, K, F, HID, NCLS = 32, 32, 2048, 2, 47
NCORES = 8
BL = B // NCORES          # batches per core
NT = F // 128             # 16 g-tiles
NP = NT // 2              # 8 fp8 DoubleRow pairs
SC = 4096.0               # hd scale for fp8 range
BN_N = float(B * F)

_CACHE = {}


def _build():
    import concourse.bass as bass
    import concourse.tile as tile
    from concourse import bacc, mybir

    f32 = mybir.dt.float32
    f16 = mybir.dt.bfloat16
    f8 = mybir.dt.float8e4
    AF = mybir.ActivationFunctionType
    OP = mybir.AluOpType
    AX = mybir.AxisListType
    DR = mybir.MatmulPerfMode.DoubleRow
    ARS = AF.Abs_reciprocal_sqrt

    nc = bacc.Bacc("TRN2", target_bir_lowering=False, debug=False,
                   num_devices=NCORES)

    def din(name, shape):
        return nc.dram_tensor(name, shape, f32, kind="ExternalInput").ap()

    x_d = din("x", [BL, F])
    nb_d = din("nb", [BL, K, F])
    w1_d = din("w1", [HID])
    b1_d = din("b1", [HID])
    g1_d = din("g1", [HID])
    be1_d = din("be1", [HID])
    w2_d = din("w2", [HID, HID])
    b2_d = din("b2", [HID])
    g2_d = din("g2", [HID])
    be2_d = din("be2", [HID])
    Wc_d = din("Wc", [NCLS, HID * F])
    bc_d = din("bc", [NCLS])
    out_d = nc.dram_tensor("out", [BL, NCLS], f32, kind="ExternalOutput").ap()

    xst_d = nc.dram_tensor("xst", [BL, F], f16).ap()     # x rows in bf16
    yst_d = nc.dram_tensor("yst", [BL, F], f16).ap()     # y rows in bf16
    cc1i = nc.dram_tensor("cc1i", [HID, 8], f32).ap()
    cc1o = nc.dram_tensor("cc1o", [HID, 8], f32, addr_space="Shared").ap()
    cc2i = nc.dram_tensor("cc2i", [HID, 8], f32).ap()
    cc2o = nc.dram_tensor("cc2o", [HID, 8], f32, addr_space="Shared").ap()
    RG = [list(range(NCORES))]

    from contextlib import ExitStack

    with tile.TileContext(nc, trace_sim=False) as tc, ExitStack() as ctx:
        P = ctx.enter_context(tc.tile_pool(name="persist", bufs=1))

        # ---- replicated params ----
        w1r = P.tile([128, HID], f32, tag="w1r")
        nc.sync.dma_start(w1r[:], w1_d[None, :].broadcast_to([128, HID]))
        b1r = P.tile([128, HID], f32, tag="b1r")
        nc.sync.dma_start(b1r[:], b1_d[None, :].broadcast_to([128, HID]))
        w2s = {}
        b2s = []
        for c in range(HID):
            for i in range(HID):
                t = P.tile([128, 1], f32, tag=f"w2s{c}{i}", name=f"w2s{c}{i}")
                nc.sync.dma_start(t[:], w2_d[c:c + 1, i:i + 1]
                                  .broadcast_to([128, 1]))
                w2s[(c, i)] = t
            t = P.tile([128, 1], f32, tag=f"b2s{c}", name=f"b2s{c}")
            nc.sync.dma_start(t[:], b2_d[c:c + 1][None, :]
                              .broadcast_to([128, 1]))
            b2s.append(t)
        def bparam(src, nm):
            out = []
            for c in range(HID):
                t = P.tile([128, 1], f32, tag=f"bp_{nm}{c}",
                           name=f"bp_{nm}{c}")
                nc.sync.dma_start(t[:], src[c:c + 1][None, :]
                                  .broadcast_to([128, 1]))
                out.append(t)
            return out

        g1b = bparam(g1_d, "g1")
        be1b = bparam(be1_d, "be1")
        g2b = bparam(g2_d, "g2")
        be2b = bparam(be2_d, "be2")
        idt8 = P.tile([8, 8], f32, tag="idt8")
        iota8 = P.tile([8, 8], mybir.dt.int32, tag="iota8")
        nc.gpsimd.iota(iota8[:], pattern=[[-1, 8]], base=0,
                       channel_multiplier=1)
        nc.vector.tensor_scalar(idt8[:], iota8[:], 0, None, op0=OP.is_equal)
        idt47 = P.tile([NCLS, NCLS], f32, tag="idt47")
        iota47 = P.tile([NCLS, NCLS], mybir.dt.int32, tag="iota47")
        nc.gpsimd.iota(iota47[:], pattern=[[-1, NCLS]], base=0,
                       channel_multiplier=1)
        nc.vector.tensor_scalar(idt47[:], iota47[:], 0, None, op0=OP.is_equal)
        ones16 = P.tile([K, 1], f16, tag="ones16")
        nc.gpsimd.memset(ones16[:], 1.0)
        epsb = P.tile([128, 1], f32, tag="epsb")
        nc.gpsimd.memset(epsb[:], 1e-30)

        # ---- per-batch persistent tiles ----
        xP, yP32, uB, rds, dps = [], [], [], [], []
        hd1a, hd2w8 = [], []
        sums1, sqs1, sums2, sqs2 = [], [], [], []
        for b in range(BL):
            xP.append(P.tile([128, NT], f32, tag=f"xP{b}", name=f"xP{b}"))
            yP32.append(P.tile([128, NT], f32, tag=f"yP32{b}", name=f"yP32{b}"))
            uB.append(P.tile([128, NT], f32, tag=f"uB{b}", name=f"uB{b}"))
            rds.append(P.tile([128, NT], f32, tag=f"rds{b}", name=f"rds{b}"))
            dps.append(P.tile([128, NT], f32, tag=f"dps{b}", name=f"dps{b}"))
            hd1a.append(P.tile([128, 2 * NT], f16, tag=f"hd1a{b}",
                               name=f"hd1a{b}"))
            hd2w8.append(P.tile([128, 64 * NP], f8, tag=f"hd2w8{b}",
                                name=f"hd2w8{b}"))
            nc.gpsimd.memset(hd2w8[b][:], 0.0)
            sums1.append(P.tile([HID, 1], f32, tag=f"sum1{b}", name=f"sum1{b}"))
            sqs1.append(P.tile([HID, 1], f32, tag=f"sq1{b}", name=f"sq1{b}"))
            sums2.append(P.tile([HID, 1], f32, tag=f"sum2{b}", name=f"sum2{b}"))
            sqs2.append(P.tile([HID, 1], f32, tag=f"sq2{b}", name=f"sq2{b}"))
        h1sA = P.tile([2 * BL, F], f32, tag="h1sA")
        h2sA = P.tile([2 * BL, F], f32, tag="h2sA")
        stats = P.tile([HID, 8], f32, tag="stats")
        stats2 = P.tile([HID, 8], f32, tag="stats2")

        small = ctx.enter_context(tc.tile_pool(name="small", bufs=4))
        s8ctx = ExitStack()
        s8pool = s8ctx.enter_context(tc.tile_pool(name="s8pool", bufs=1))
        S8 = [s8pool.tile([128, NT * F], f8, tag=f"S8_{b}", name=f"S8_{b}")
              for b in range(BL)]

        # ---- phase 0: rows, ysum, per-partition scalars ----
        with tc.tile_pool(name="yps", bufs=1, space="PSUM") as ypsp, \
             tc.tile_pool(name="nbp", bufs=2) as nbp, \
             tc.tile_pool(name="yrp", bufs=2) as yrp:
            for b in range(BL):
                nc.gpsimd.dma_start(xst_d[b:b + 1, :], x_d[b:b + 1, :])
                nc.sync.dma_start(
                    xP[b][:],
                    x_d[b:b + 1, :].rearrange("one (t p) -> (one p) t", p=128))
                nb16 = nbp.tile([K, F], f16, tag="nb16")
                nc.gpsimd.dma_start(nb16[:], nb_d[b])
                yp = ypsp.tile([1, F], f32, tag="yp")
                for c in range(4):
                    nc.tensor.matmul(yp[:, c * 512:(c + 1) * 512], ones16[:],
                                     nb16[:, c * 512:(c + 1) * 512],
                                     start=True, stop=True)
                yrow = yrp.tile([1, F], f16, tag="yrow")
                nc.scalar.copy(yrow[:], yp[:])
                nc.sync.dma_start(yst_d[b:b + 1, :], yrow[:])
                yP16 = yrp.tile([128, NT], f16, tag="yP16")
                nc.sync.dma_start(
                    yP16[:],
                    yst_d[b:b + 1, :].rearrange("one (t p) -> (one p) t",
                                                p=128))
                # y_safe = y + 1e-6: avoids exact-zero division; perturbs
                # a[g,f] by 1e-6*x_f only (negligible).
                nc.vector.tensor_scalar(yP32[b][:], yP16[:], 1e-6, None,
                                        op0=OP.add)
                yinv = yrp.tile([128, NT], f32, tag="yinv")
                yscr = yrp.tile([128, NT], f32, tag="yscr")
                nc.vector.reciprocal_approx_accurate(yinv[:], yP32[b][:],
                                                     yscr[:])
                nc.vector.tensor_mul(uB[b][:], xP[b][:], yinv[:])

        # ---- phase 1: layer-1 fused generate+contract; fp8 cast ----
        with tc.tile_pool(name="otp", bufs=2, space="PSUM") as otp, \
             tc.tile_pool(name="xyb", bufs=1) as xyb, \
             tc.tile_pool(name="inp", bufs=2) as inp, \
             tc.tile_pool(name="rp", bufs=2) as rp, \
             tc.tile_pool(name="s16p", bufs=3) as s16p, \
             tc.tile_pool(name="hsbp", bufs=2) as hsbp, \
             tc.tile_pool(name="absp", bufs=1) as absp:
            for b in range(BL):
                xBb = xyb.tile([128, F], f16, tag="xBb")
                nc.sync.dma_start(xBb[:],
                                  xst_d[b:b + 1, :].broadcast_to([128, F]))
                yBb = xyb.tile([128, F], f16, tag="yBb")
                nc.sync.dma_start(yBb[:],
                                  yst_d[b:b + 1, :].broadcast_to([128, F]))
                outT = otp.tile([HID, F], f32, tag="outT")
                for t in range(NT):
                    inner = inp.tile([128, F], f16, tag="inner")
                    nc.vector.scalar_tensor_tensor(
                        inner[:], yBb[:], uB[b][:, t:t + 1], xBb[:],
                        op0=OP.mult, op1=OP.add)
                    r = rp.tile([128, F], f16, tag="r")
                    nc.scalar.activation(r[:], inner[:], ARS,
                                         bias=epsb[:],
                                         scale=yP32[b][:, t:t + 1])
                    S16 = s16p.tile([128, F], f16, tag="S16")
                    nc.vector.scalar_tensor_tensor(
                        S16[:], inner[:], yP32[b][:, t:t + 1], r[:],
                        op0=OP.mult, op1=OP.mult)
                    # |S| + rowsum(|S|) in one op.
                    # Output overwrites the dead `inner` tile (scratch).
                    nc.vector.scalar_tensor_tensor(
                        inner[:], S16[:], -1.0, S16[:],
                        op0=OP.mult, op1=OP.max,
                        accum_out=dps[b][:, t:t + 1])
                    # fp8 resident copy for layer 2
                    nc.gpsimd.dma_start(S8[b][:, t * F:(t + 1) * F], S16[:])
                    # rds col: 4096 / (denom + 1e-7)
                    dne = small.tile([128, 1], f32, tag="dne")
                    nc.vector.tensor_scalar_add(dne[:], dps[b][:, t:t + 1],
                                                1e-7)
                    rcp = small.tile([128, 1], f32, tag="rcp")
                    nc.vector.reciprocal(rcp[:], dne[:])
                    nc.vector.tensor_scalar(rds[b][:, t:t + 1], rcp[:], SC,
                                            None, op0=OP.mult)
                    # hd1 col pair (scaled): ((x*w1+b1) * rds)
                    hraw = small.tile([128, HID], f32, tag="hraw")
                    nc.vector.scalar_tensor_tensor(
                        hraw[:], w1r[:], xP[b][:, t:t + 1], b1r[:],
                        op0=OP.mult, op1=OP.add)
                    nc.vector.tensor_scalar(
                        hd1a[b][:, 2 * t:2 * t + 2], hraw[:],
                        rds[b][:, t:t + 1], None, op0=OP.mult)
                    # L1 contraction (bf16)
                    for c in range(4):
                        fo = c * 512
                        nc.tensor.matmul(
                            outT[:, fo:fo + 512],
                            hd1a[b][:, 2 * t:2 * t + 2],
                            S16[:, fo:fo + 512],
                            start=(t == 0), stop=(t == NT - 1))
                # evacuate (rescale) + stats; cast-DMA into stacked h1sA
                hsb = hsbp.tile([HID, F], f16, tag="hsb")
                nc.scalar.activation(hsb[:], outT[:], AF.Copy,
                                     scale=1.0 / SC, accum_out=sums1[b][:])
                sqscr = absp.tile([HID, F], f16, tag="sqscr")
                nc.vector.scalar_tensor_tensor(
                    sqscr[:], hsb[:], 1.0, hsb[:],
                    op0=OP.mult, op1=OP.mult, accum_out=sqs1[b][:])
                nc.gpsimd.dma_start(h1sA[2 * b:2 * b + 2, :], hsb[:])

            # ---- stats allreduce #1 ----
            nc.gpsimd.memset(stats[:], 0.0)
            t01 = small.tile([HID, 1], f32, tag="t01")
            t23 = small.tile([HID, 1], f32, tag="t23")
            nc.vector.tensor_add(t01[:], sums1[0][:], sums1[1][:])
            nc.vector.tensor_add(t23[:], sums1[2][:], sums1[3][:])
            nc.vector.tensor_add(stats[:, 0:1], t01[:], t23[:])
            nc.vector.tensor_add(t01[:], sqs1[0][:], sqs1[1][:])
            nc.vector.tensor_add(t23[:], sqs1[2][:], sqs1[3][:])
            nc.vector.tensor_add(stats[:, 1:2], t01[:], t23[:])
            nc.sync.dma_start(cc1i, stats[:])
            nc.gpsimd.collective_compute(
                "AllReduce", OP.add, replica_groups=RG,
                ins=[cc1i], outs=[cc1o])

        def bn_coeffs_bcast(cco, gb, beb_, tag):
            # per-channel BN affine coefficients, replicated on 128
            # partitions straight from the allreduce DRAM result
            alb, beb = [], []
            for c in range(HID):
                ab = small.tile([128, 2], f32, tag=f"ab{tag}{c}")
                nc.sync.dma_start(ab[:], cco[c:c + 1, 0:2]
                                  .broadcast_to([128, 2]))
                mu = small.tile([128, 1], f32, tag=f"mu{tag}{c}")
                nc.vector.tensor_scalar(mu[:], ab[:, 0:1], 1.0 / BN_N, None,
                                        op0=OP.mult)
                vare = small.tile([128, 1], f32, tag=f"vr{tag}{c}")
                # ex2 - mu^2 + eps = (ab1/N) - mu*mu + eps
                nc.vector.tensor_scalar(vare[:], ab[:, 1:2], 1.0 / BN_N, None,
                                        op0=OP.mult)
                mm = small.tile([128, 1], f32, tag=f"mm{tag}{c}")
                nc.vector.tensor_mul(mm[:], mu[:], mu[:])
                nc.vector.tensor_sub(vare[:], vare[:], mm[:])
                nc.vector.tensor_scalar_add(vare[:], vare[:], 1e-5)
                ivs = small.tile([128, 1], f32, tag=f"iv{tag}{c}")
                nc.scalar.activation(ivs[:], vare[:], ARS)
                al = P.tile([128, 1], f32, tag=f"al{tag}{c}",
                            name=f"al{tag}{c}")
                nc.vector.tensor_mul(al[:], gb[c][:], ivs[:])
                am = small.tile([128, 1], f32, tag=f"am{tag}{c}")
                nc.vector.tensor_mul(am[:], al[:], mu[:])
                be = P.tile([128, 1], f32, tag=f"bt{tag}{c}",
                            name=f"bt{tag}{c}")
                nc.vector.tensor_sub(be[:], beb_[c][:], am[:])
                alb.append(al)
                beb.append(be)
            return alb, beb

        def transpose_bn_softsign(hsA, alb, beb, Apool, tag):
            # [8, F] -> A [128, NT*8] cols (t, 2b+c), BN affine + softsign
            A = Apool.tile([128, NT * 8], f32, tag=f"A{tag}")
            with tc.tile_pool(name=f"tp{tag}", bufs=2, space="PSUM") as tpp:
                for t in range(NT):
                    tp = tpp.tile([128, 8], f32, tag="tp")
                    nc.tensor.transpose(tp[:], hsA[:, t * 128:(t + 1) * 128],
                                        idt8[:])
                    for c in range(HID):
                        nc.vector.tensor_scalar(
                            A[:, t * 8 + c:t * 8 + 8:2], tp[:, c:8:2],
                            alb[c][:], beb[c][:], op0=OP.mult, op1=OP.add)
            # softsign: A = A / (1 + |A|)
            ab = Apool.tile([128, NT * 8], f32, tag=f"ab{tag}")
            nc.vector.scalar_tensor_tensor(ab[:], A[:], -1.0, A[:],
                                           op0=OP.mult, op1=OP.max)
            nc.vector.tensor_scalar_add(ab[:], ab[:], 1.0)
            rec = Apool.tile([128, NT * 8], f32, tag=f"rec{tag}")
            nc.vector.reciprocal(rec[:], ab[:])
            nc.vector.tensor_mul(A[:], A[:], rec[:])
            return A

        # ---- phase 3: BN1 + softsign + hd2 (fp8 cells) ----
        alb1, beb1 = bn_coeffs_bcast(cc1o, g1b, be1b, "1")
        with tc.tile_pool(name="ap3", bufs=1) as ap3:
            A1 = transpose_bn_softsign(h1sA, alb1, beb1, ap3, "1")
            for b in range(BL):
                a0 = A1[:, 2 * b + 0::8]
                a1 = A1[:, 2 * b + 1::8]
                for c in range(HID):
                    m1 = small.tile([128, NT], f32, tag="m1")
                    nc.vector.tensor_scalar(m1[:], a1, w2s[(c, 1)][:], None,
                                            op0=OP.mult)
                    qq = small.tile([128, NT], f32, tag="qq")
                    nc.vector.scalar_tensor_tensor(
                        qq[:], a0, w2s[(c, 0)][:], m1[:],
                        op0=OP.mult, op1=OP.add)
                    hd2f = small.tile([128, NT], f32, tag="hd2f")
                    nc.vector.scalar_tensor_tensor(
                        hd2f[:], qq[:], b2s[c][:], rds[b][:],
                        op0=OP.add, op1=OP.mult)
                    # scatter into fp8 DoubleRow cells: byte 64u + 32c + 16j
                    dst = hd2w8[b][:, :].rearrange(
                        "p (u r) -> p u r", r=64)[:, :, 32 * c:32 * c + 17:16]
                    src = hd2f[:, :].rearrange("p (u j) -> p u j", u=NP)
                    nc.vector.tensor_copy(dst, src)

        # ---- phase 4: layer 2 (fp8 DoubleRow from resident S8) ----
        with tc.tile_pool(name="s4scr", bufs=2) as s4scr, \
             tc.tile_pool(name="otp2", bufs=2, space="PSUM") as otp2:
            for b in range(BL):
                outT2 = otp2.tile([HID, F], f32, tag="outT")
                S8v = S8[b][:, :].rearrange("p (t f) -> p t f", t=NT)
                for u in range(NP):
                    lhsT = hd2w8[b][:, 64 * u:64 * (u + 1)].rearrange(
                        "p (m r) -> p r m", r=32)[:, 0:32:16, :]
                    for c in range(4):
                        fo = c * 512
                        nc.tensor.matmul(
                            outT2[:, fo:fo + 512], lhsT,
                            S8v[:, 2 * u:2 * u + 2, fo:fo + 512],
                            start=(u == 0), stop=(u == NP - 1),
                            perf_mode=DR)
                hsb2 = s4scr.tile([HID, F], f16, tag="hsb2")
                nc.scalar.activation(hsb2[:], outT2[:], AF.Copy,
                                     scale=1.0 / SC, accum_out=sums2[b][:])
                sqscr2 = s4scr.tile([HID, F], f16, tag="sqscr2")
                nc.vector.scalar_tensor_tensor(
                    sqscr2[:], hsb2[:], 1.0, hsb2[:],
                    op0=OP.mult, op1=OP.mult, accum_out=sqs2[b][:])
                nc.gpsimd.dma_start(h2sA[2 * b:2 * b + 2, :], hsb2[:])

            nc.gpsimd.memset(stats2[:], 0.0)
            u01 = small.tile([HID, 1], f32, tag="u01")
            u23 = small.tile([HID, 1], f32, tag="u23")
            nc.vector.tensor_add(u01[:], sums2[0][:], sums2[1][:])
            nc.vector.tensor_add(u23[:], sums2[2][:], sums2[3][:])
            nc.vector.tensor_add(stats2[:, 0:1], u01[:], u23[:])
            nc.vector.tensor_add(u01[:], sqs2[0][:], sqs2[1][:])
            nc.vector.tensor_add(u23[:], sqs2[2][:], sqs2[3][:])
            nc.vector.tensor_add(stats2[:, 1:2], u01[:], u23[:])
            nc.sync.dma_start(cc2i, stats2[:])
            nc.gpsimd.collective_compute(
                "AllReduce", OP.add, replica_groups=RG,
                ins=[cc2i], outs=[cc2o])

        s8ctx.close()  # release 128KB/partition of fp8 S before classifier

        # ---- phase 6: BN2 + softsign + classifier ----
        alb2, beb2 = bn_coeffs_bcast(cc2o, g2b, be2b, "2")
        with tc.tile_pool(name="ap6", bufs=1) as ap6, \
             tc.tile_pool(name="wstg", bufs=1) as wstg, \
             tc.tile_pool(name="wtp", bufs=2, space="PSUM") as wtp, \
             tc.tile_pool(name="clsp", bufs=1, space="PSUM") as clsp:
            H = transpose_bn_softsign(h2sA, alb2, beb2, ap6, "2")
            Wstage = wstg.tile([NCLS, HID * F], f32, tag="Wstage")
            nc.sync.dma_start(Wstage[:], Wc_d)
            WcT = wstg.tile([128, 32 * NCLS], f32, tag="WcT")
            for m in range(32):
                wps = wtp.tile([128, NCLS], f32, tag="wps")
                nc.tensor.transpose(wps[:], Wstage[:, m * 128:(m + 1) * 128],
                                    idt47[:])
                nc.vector.tensor_copy(WcT[:, m * NCLS:(m + 1) * NCLS], wps[:])
            cls = clsp.tile([BL, NCLS], f32, tag="cls")
            for m in range(32):
                c, t = m // NT, m % NT
                lhsT = H[:, t * 8 + c:t * 8 + 8:2]
                nc.tensor.matmul(cls[:], lhsT, WcT[:, m * NCLS:(m + 1) * NCLS],
                                 start=(m == 0), stop=(m == 31))
            bcr = wstg.tile([BL, NCLS], f32, tag="bcr")
            nc.sync.dma_start(bcr[:], bc_d[None, :].broadcast_to([BL, NCLS]))
            ob = wstg.tile([BL, NCLS], f32, tag="ob")
            nc.vector.tensor_add(ob[:], cls[:], bcr[:])
            nc.sync.dma_start(out_d, ob[:])

    nc.compile()
    return nc


def _get_nc():
    if "nc" not in _CACHE:
        _CACHE["nc"] = _build()
    return _CACHE["nc"]


def kernel(**inputs):
    from concourse.bass_utils import run_bass_kernel_spmd

    nc = _get_nc()
    x = np.ascontiguousarray(np.asarray(inputs["x"], np.float32)[:, 0, :])
    nb = np.ascontiguousarray(
        np.asarray(inputs["neighbor"], np.float32)[:, :, 0, :])
    w1 = np.ascontiguousarray(np.asarray(inputs["w1"], np.float32)[:, 0])
    base = {
        "w1": w1,
        "b1": np.ascontiguousarray(np.asarray(inputs["b1"], np.float32)),
        "g1": np.ascontiguousarray(np.asarray(inputs["g1"], np.float32)),
        "be1": np.ascontiguousarray(np.asarray(inputs["beta1"], np.float32)),
        "w2": np.ascontiguousarray(np.asarray(inputs["w2"], np.float32)),
        "b2": np.ascontiguousarray(np.asarray(inputs["b2"], np.float32)),
        "g2": np.ascontiguousarray(np.asarray(inputs["g2"], np.float32)),
        "be2": np.ascontiguousarray(np.asarray(inputs["beta2"], np.float32)),
        "Wc": np.ascontiguousarray(np.asarray(inputs["Wc"], np.float32)),
        "bc": np.ascontiguousarray(np.asarray(inputs["bc"], np.float32)),
    }
    in_maps = []
    for i in range(NCORES):
        m = dict(base)
        m["x"] = np.ascontiguousarray(x[i * BL:(i + 1) * BL])
        m["nb"] = np.ascontiguousarray(nb[i * BL:(i + 1) * BL])
        in_maps.append(m)

    trace = bool(int(os.environ.get("KERNEL_TRACE", "0")))
    res = run_bass_kernel_spmd(nc, in_maps, list(range(NCORES)), trace=trace)
    _CACHE["last_exec_time_ns"] = getattr(res, "exec_time_ns", None)
    _CACHE["last_results"] = res
    out = np.concatenate([res.results[i]["out"] for i in range(NCORES)],
                         axis=0)
    return out.astype(np.float32)
